# revision 3
# baseline (speedup 1.0000x reference)
"""NeRF MLP forward on 8 TRN2 NeuronCores — Bass/Tile kernel.

Strategy (pure data parallel, hardcoded):
  N=131072 rows split 8 ways (16384/core), weights replicated.

Host-side preprocessing:
  * posenc is algebraically rewritten: all sin/cos arguments are range-reduced to
    [-pi, pi] on the host (fp64), and cos(y) is computed as sin(pi/2 - |z|).
    The device does ONE Sin activation per batch tile with per-partition
    scale (+1/-1/0) and bias (0 / pi/2).
  * raw xyz/d passthrough features become pseudo-frequency rows sin(2^-10 x)
    whose W columns are scaled by 2^10 (error ~1e-7 relative).
  * "ones" rows in the embedding provide fc1/fc6/fc10 biases as weight rows.
  * weights are pre-transposed/permuted and packed into one [128, C] array.

Device per batch tile (nb=512, 32 tiles/core):
  Sin -> fc1..fc11 as float32r matmuls (full PE speed, ~12-bit mantissa)
  accumulating in PSUM fp32, bias+relu fused on DVE/ACT, sigmoid/density at the end.
"""
import sys
for _p in ("/opt/trn_rl_repo", "/root/.axon_site/_ro/trn_rl_repo"):
    if _p not in sys.path:
        sys.path.insert(0, _p)

import numpy as np

N = 131072
CORES = 8
NC = N // CORES          # 16384 rows per core
NB = 512                 # batch tile (psum bank)
T = NC // NB             # 32 tiles per core
ZCHUNK = 2048            # Z prefetch granularity
OCHUNK = 2048            # output staging granularity
L_POS, L_DIR, H = 10, 4, 256
PSEUDO = 2.0 ** -10      # pseudo frequency for raw passthrough features
ZR = 96                  # Z/emb rows

_CACHE = {}


def _emb_maps():
    """Row maps for the embedding tile.

    x-block rows 0:64  (fc1 / fc6 K-chunk), d-block rows 64:96 (fc10 K-chunk).
    Returns (x_perm, x_scale, d_perm, d_scale): per-row source column in the
    original posenc feature order and the weight-column scale factor.
    """
    x_perm = np.zeros(63, np.int64)
    x_scale = np.ones(63, np.float64)
    for i in range(3):
        x_perm[i] = i
        x_scale[i] = 1.0 / PSEUDO
    for l in range(L_POS):
        for i in range(3):
            x_perm[3 + 3 * l + i] = 3 + 6 * l + i        # sin
            x_perm[33 + 3 * l + i] = 6 + 6 * l + i       # cos
    d_perm = np.zeros(27, np.int64)
    d_scale = np.ones(27, np.float64)
    for i in range(3):
        d_perm[i] = i
        d_scale[i] = 1.0 / PSEUDO
    for l in range(L_DIR):
        for i in range(3):
            d_perm[3 + 3 * l + i] = 3 + 6 * l + i
            d_perm[15 + 3 * l + i] = 6 + 6 * l + i
    return x_perm, x_scale, d_perm, d_scale


def _host_z(xyz, d):
    """Build the [ZR, N] range-reduced posenc input array (fp32)."""
    x64 = xyz.astype(np.float64).T    # (3, N)
    d64 = d.astype(np.float64).T
    Z = np.zeros((ZR, xyz.shape[0]), np.float64)

    def fill(base, v64, L):
        Z[base + 0:base + 3] = v64 * PSEUDO                    # pseudo raw (sin arg)
        for l in range(L):
            y = v64 * (2.0 ** l)
            z = y - 2.0 * np.pi * np.round(y / (2.0 * np.pi))
            Z[base + 3 + 3 * l: base + 6 + 3 * l] = z          # sin rows
            Z[base + 3 + 3 * L + 3 * l: base + 6 + 3 * L + 3 * l] = np.abs(z)  # cos rows

    fill(0, x64, L_POS)     # rows 0:63 (63 = ones row)
    fill(64, d64, L_DIR)    # rows 64:91 (91 = ones row)
    return Z.astype(np.float32)


def _posenc_consts():
    """Per-partition (scale, bias) for the Sin activation: [ZR, 2] fp32."""
    sc = np.zeros(ZR, np.float64)
    bi = np.zeros(ZR, np.float64)

    def fill(base, L):
        sc[base:base + 3] = 1.0                      # pseudo raw: sin(z)
        sc[base + 3:base + 3 + 3 * L] = 1.0          # sin rows
        sc[base + 3 + 3 * L:base + 3 + 6 * L] = -1.0  # cos rows: sin(pi/2 - |z|)
        bi[base + 3 + 3 * L:base + 3 + 6 * L] = np.pi / 2
        sc[base + 3 + 6 * L] = 0.0                   # ones row
        bi[base + 3 + 6 * L] = np.pi / 2

    fill(0, L_POS)
    fill(64, L_DIR)
    return np.stack([sc, bi], axis=1).astype(np.float32)


def _pack_weights(inp):
    """Pack all (permuted, transposed, padded) weights into one [128, C] fp32
    array. Returns (wpack, cols) where cols maps name -> (col0, width)."""
    x_perm, x_scale, d_perm, d_scale = _emb_maps()
    blocks, cols = [], {}

    def add(name, arr, row0=0):
        a = np.zeros((128, arr.shape[1]), np.float32)
        a[row0:row0 + arr.shape[0]] = arr
        cols[name] = (sum(b.shape[1] for b in blocks), arr.shape[1])
        blocks.append(a)

    W1, b1 = inp["W1"], inp["b1"]                     # (256, 63), (256,)
    w1t = np.zeros((64, 256), np.float64)
    w1t[:63] = W1.astype(np.float64).T[x_perm] * x_scale[:, None]
    w1t[63] = b1
    add("w1", w1t.astype(np.float32))

    for n in (2, 3, 4, 5, 7, 8):
        Wt = inp[f"W{n}"].astype(np.float32).T        # (256, 256)
        add(f"w{n}k0", Wt[0:128])
        add(f"w{n}k1", Wt[128:256])

    W6, b6 = inp["W6"], inp["b6"]                     # (256, 319)
    W6t = W6.astype(np.float64).T
    add("w6a0", W6t[0:128].astype(np.float32))
    add("w6a1", W6t[128:256].astype(np.float32))
    w6b = np.zeros((64, 256), np.float64)
    w6b[:63] = W6t[256:319][x_perm] * x_scale[:, None]
    w6b[63] = b6
    add("w6b", w6b.astype(np.float32))

    W9t = inp["W9"].astype(np.float32).T              # (256, 257)
    add("w9k0", W9t[0:128])
    add("w9k1", W9t[128:256])

    W10, b10 = inp["W10"], inp["b10"]                 # (128, 283)
    W10t = W10.astype(np.float64).T                   # (283, 128)
    add("w10a0", W10t[0:128].astype(np.float32))
    add("w10a1", W10t[128:256].astype(np.float32))
    w10c = np.zeros((32, 128), np.float64)
    w10c[:27] = W10t[256:283][d_perm] * d_scale[:, None]
    w10c[27] = b10
    add("w10c", w10c.astype(np.float32), row0=64)     # partitions 64:96 (match emb[64:96])

    add("w11", inp["W11"].astype(np.float32).T)       # (128, 3)

    wpack = np.concatenate(blocks, axis=1)
    return wpack, cols


def _pack_biases(inp):
    """[128, 18] fp32: col0 posenc scale, col1 posenc bias, then per-layer biases."""
    bp = np.zeros((128, 18), np.float32)
    bp[0:ZR, 0:2] = _posenc_consts()
    c = 2
    bcols = {}
    for n in (2, 3, 4, 5, 7, 8, 9):
        b = inp[f"b{n}"].astype(np.float32)
        bp[:, c] = b[0:128]
        bp[:, c + 1] = b[128:256]
        bcols[n] = c
        c += 2
    bp[0, 16] = np.float32(inp["b9"][256])            # density bias
    bp[0:3, 17] = inp["b11"].astype(np.float32)
    return bp, bcols


def _build_program():
    import concourse.bass as bass
    from concourse import bacc
    import concourse.tile as tile
    import concourse.mybir as mybir
    from contextlib import ExitStack

    dt = mybir.dt
    AF = mybir.ActivationFunctionType
    OP = mybir.AluOpType
    WCOLS = _CACHE["wcols"]
    BCOLS = _CACHE["bcols"]
    WNC = _CACHE["wpack_cols"]

    nc = bacc.Bacc("TRN2", target_bir_lowering=False, debug=False)
    Zd = nc.dram_tensor("Z", [ZR, NC], dt.float32, kind="ExternalInput").ap()
    Wd = nc.dram_tensor("WP", [128, WNC], dt.float32, kind="ExternalInput").ap()
    Bd = nc.dram_tensor("BP", [128, 18], dt.float32, kind="ExternalInput").ap()
    col_d = nc.dram_tensor("colorT", [3, NC], dt.float32, kind="ExternalOutput").ap()
    den_d = nc.dram_tensor("density", [1, NC], dt.float32, kind="ExternalOutput").ap()

    with tile.TileContext(nc) as tc, ExitStack() as ctx:
        wpool = ctx.enter_context(tc.tile_pool(name="wpool", bufs=1))
        zpool = ctx.enter_context(tc.tile_pool(name="zpool", bufs=2))
        epool = ctx.enter_context(tc.tile_pool(name="epool", bufs=3))
        apool = ctx.enter_context(tc.tile_pool(name="apool", bufs=2))
        opool = ctx.enter_context(tc.tile_pool(name="opool", bufs=2))
        pwide = ctx.enter_context(tc.tile_pool(name="pwide", bufs=2, space="PSUM"))
        phalf = ctx.enter_context(tc.tile_pool(name="phalf", bufs=2, space="PSUM"))
        ptiny = ctx.enter_context(tc.tile_pool(name="ptiny", bufs=1, space="PSUM"))

        wst = wpool.tile([128, WNC], dt.float32, tag="wst")
        nc.gpsimd.dma_start(wst[:], Wd[:])
        wr = wpool.tile([128, WNC], dt.float32r, tag="wr")
        nc.vector.tensor_copy(wr[:], wst[:])
        bt = wpool.tile([128, 18], dt.float32, tag="bt")
        nc.gpsimd.dma_start(bt[:], Bd[:])

        def W(name, mslice=None, rows=None):
            c0, w = WCOLS[name]
            ap = wr[:, c0:c0 + w] if rows is None else wr[rows[0]:rows[1], c0:c0 + w]
            if mslice is not None:
                ap = ap[:, mslice[0]:mslice[1]] if rows is None else \
                    wr[rows[0]:rows[1], c0 + mslice[0]:c0 + mslice[1]]
            return ap

        def bias(n, half):
            c = BCOLS[n] + half
            return bt[0:128, c:c + 1]

        zt = None
        ct = dtile = None
        for j in range(T):
            if j % (ZCHUNK // NB) == 0:
                zt = zpool.tile([ZR, ZCHUNK], dt.float32, tag="zt")
                z0 = (j // (ZCHUNK // NB)) * ZCHUNK
                nc.sync.dma_start(zt[:], Zd[:, z0:z0 + ZCHUNK])
            off = (j % (ZCHUNK // NB)) * NB
            zs = zt[:, off:off + NB]

            if j % (OCHUNK // NB) == 0:
                ct = opool.tile([3, OCHUNK], dt.float32, tag="ct")
                dtile = opool.tile([1, OCHUNK], dt.float32, tag="dt")
            ooff = (j % (OCHUNK // NB)) * NB

            # posenc: one Sin for all 96 rows (f32r output)
            emb = epool.tile([ZR, NB], dt.float32r, tag="emb")
            nc.scalar.activation(emb[:], zs, AF.Sin,
                                 bias=bt[0:ZR, 1:2], scale=bt[0:ZR, 0:1])

            # fc1: K=64 chunk (emb[0:64]); bias via ones row; relu only
            ps1 = pwide.tile([128, 2 * NB], dt.float32, tag="pw")
            for m in range(2):
                nc.tensor.matmul(ps1[:, m * NB:(m + 1) * NB],
                                 W("w1", (m * 128, (m + 1) * 128), rows=(0, 64)),
                                 emb[0:64, :], start=True, stop=True)
            act = apool.tile([128, 2 * NB], dt.float32r, tag="act1")
            nc.scalar.activation(act[:], ps1[:], AF.Relu)

            # fc2-fc5: K=256; fused bias+relu on DVE
            for n in (2, 3, 4, 5):
                ps = [phalf.tile([128, NB], dt.float32, tag="ph", name=f"ps{n}_{m_}") for m_ in range(2)]
                for m in range(2):
                    for k in range(2):
                        nc.tensor.matmul(ps[m][:],
                                         W(f"w{n}k{k}", (m * 128, (m + 1) * 128)),
                                         act[:, k * NB:(k + 1) * NB],
                                         start=(k == 0), stop=(k == 1))
                nact = apool.tile([128, 2 * NB], dt.float32r, tag=f"act{n}")
                for m in range(2):
                    nc.vector.tensor_scalar(nact[:, m * NB:(m + 1) * NB], ps[m][:],
                                            bias(n, m), 0.0, OP.add, OP.max)
                act = nact

            # fc6: K = 256 (act5) + 64 (emb); bias via ones row; relu only
            ps6 = pwide.tile([128, 2 * NB], dt.float32, tag="pw")
            for m in range(2):
                nc.tensor.matmul(ps6[:, m * NB:(m + 1) * NB],
                                 W("w6a0", (m * 128, (m + 1) * 128)),
                                 act[:, 0:NB], start=True, stop=False)
                nc.tensor.matmul(ps6[:, m * NB:(m + 1) * NB],
                                 W("w6a1", (m * 128, (m + 1) * 128)),
                                 act[:, NB:2 * NB], start=False, stop=False)
                nc.tensor.matmul(ps6[:, m * NB:(m + 1) * NB],
                                 W("w6b", (m * 128, (m + 1) * 128), rows=(0, 64)),
                                 emb[0:64, :], start=False, stop=True)
            act = apool.tile([128, 2 * NB], dt.float32r, tag="act6")
            nc.scalar.activation(act[:], ps6[:], AF.Relu)

            # fc7, fc8: bias+relu on ACT
            for n in (7, 8):
                ps = [phalf.tile([128, NB], dt.float32, tag="ph", name=f"ps{n}_{m_}") for m_ in range(2)]
                for m in range(2):
                    for k in range(2):
                        nc.tensor.matmul(ps[m][:],
                                         W(f"w{n}k{k}", (m * 128, (m + 1) * 128)),
                                         act[:, k * NB:(k + 1) * NB],
                                         start=(k == 0), stop=(k == 1))
                nact = apool.tile([128, 2 * NB], dt.float32r, tag=f"act{n}")
                for m in range(2):
                    nc.scalar.activation(nact[:, m * NB:(m + 1) * NB], ps[m][:],
                                         AF.Relu, bias=bias(n, m))
                act = nact

            # fc9: 257 outputs: color 0:256 (bias, no relu), density col 256 (bias+relu)
            ps9 = [phalf.tile([128, NB], dt.float32, tag="ph", name=f"ps9_{m_}") for m_ in range(2)]
            psd = ptiny.tile([1, NB], dt.float32, tag="pd")
            for m in range(2):
                for k in range(2):
                    nc.tensor.matmul(ps9[m][:],
                                     W(f"w9k{k}", (m * 128, (m + 1) * 128)),
                                     act[:, k * NB:(k + 1) * NB],
                                     start=(k == 0), stop=(k == 1))
            for k in range(2):
                nc.tensor.matmul(psd[:], W(f"w9k{k}", (256, 257)),
                                 act[:, k * NB:(k + 1) * NB],
                                 start=(k == 0), stop=(k == 1))
            act9 = apool.tile([128, 2 * NB], dt.float32r, tag="act9")
            for m in range(2):
                nc.vector.tensor_scalar_add(act9[:, m * NB:(m + 1) * NB], ps9[m][:],
                                            bias(9, m))
            nc.vector.tensor_scalar(dtile[0:1, ooff:ooff + NB], psd[:],
                                    bt[0:1, 16:17], 0.0, OP.add, OP.max)

            # fc10: K = 256 (act9) + 32 (emb d-block at partitions 64:96); relu only
            ps10 = phalf.tile([128, NB], dt.float32, tag="ph")
            nc.tensor.matmul(ps10[:], W("w10a0"), act9[:, 0:NB],
                             start=True, stop=False)
            nc.tensor.matmul(ps10[:], W("w10a1"), act9[:, NB:2 * NB],
                             start=False, stop=False)
            nc.tensor.matmul(ps10[:], W("w10c", rows=(64, 96)),
                             emb[64:96, :], start=False, stop=True)
            act10 = apool.tile([128, NB], dt.float32r, tag="act10")
            nc.scalar.activation(act10[:], ps10[:], AF.Relu)

            # fc11 + sigmoid
            ps11 = ptiny.tile([3, NB], dt.float32, tag="pc")
            nc.tensor.matmul(ps11[:], W("w11"), act10[:], start=True, stop=True)
            nc.scalar.activation(ct[0:3, ooff:ooff + NB], ps11[:], AF.Sigmoid,
                                 bias=bt[0:3, 17:18])

            if (j + 1) % (OCHUNK // NB) == 0:
                o0 = (j // (OCHUNK // NB)) * OCHUNK
                nc.sync.dma_start(col_d[:, o0:o0 + OCHUNK], ct[:])
                nc.sync.dma_start(den_d[:, o0:o0 + OCHUNK], dtile[:])

    nc.compile()
    return nc


def _prepare(inputs):
    inputs = {k: np.asarray(v) for k, v in inputs.items()}
    wpack, wcols = _pack_weights(inputs)
    bpack, bcols = _pack_biases(inputs)
    _CACHE["wcols"] = wcols
    _CACHE["bcols"] = bcols
    _CACHE["wpack_cols"] = wpack.shape[1]
    return inputs, wpack, bpack


def kernel(**inputs):
    from concourse.bass_utils import run_bass_kernel_spmd

    inputs, wpack, bpack = _prepare(inputs)
    if "nc" not in _CACHE:
        _CACHE["nc"] = _build_program()
    nc = _CACHE["nc"]

    Z = _host_z(inputs["xyz"], inputs["d"])
    in_maps = [{"Z": np.ascontiguousarray(Z[:, c * NC:(c + 1) * NC]),
                "WP": wpack, "BP": bpack} for c in range(CORES)]
    res = run_bass_kernel_spmd(nc, in_maps, core_ids=list(range(CORES)))
    color = np.concatenate([res.results[c]["colorT"] for c in range(CORES)],
                           axis=1).T.astype(np.float32)
    density = np.concatenate([res.results[c]["density"][0] for c in range(CORES)]
                             ).astype(np.float32)
    return np.ascontiguousarray(color), density
